# revision 41
# baseline (speedup 1.0000x reference)
"""Paged-attention decode (GQA, B=32, H=32, KVH=8, HD=128) on 8 TRN2 NeuronCores.

Sharding: tensor-parallel over KV heads (8 KV heads -> 8 cores). Each core
handles one KV head and its 4 query heads for ALL 32 sequences, so the
per-core work is identical regardless of the sequence-length distribution.

Host side (not on the device critical path):
  - gathers each sequence's valid KV-cache blocks via block_tables,
  - sorts sequences by length (desc) and packs them into 4 groups of 8,
  - packs K transposed as (HD, tokens) and V as (tokens, HD), bf16,
    round-major (one contiguous ~1 MiB DMA per 512-token round),
  - builds an additive -1e30 length mask.

Device kernel (per core, identical SPMD graph):
  for each group of 8 seqs:
    phase A: per 512-token round: 1 K DMA + 8 q@K^T matmuls -> PSUM,
             PSUM + mask -> SBUF scores
    phase B: row-max, exp (fused bias/scale/row-sum) -> p (bf16)
    phase C: per round: 1 V DMA, PE-transpose p chunks, p^T @ V accumulated
             in PSUM per sequence
    normalize by 1/row-sum and DMA out (seq*qhead, HD).
"""

from contextlib import ExitStack

import numpy as np
import ml_dtypes

import concourse.bass as bass
import concourse.tile as tile
from concourse import mybir
from concourse.bass_utils import run_bass_kernel_spmd
from concourse.vector_clock import ScopedClock

B, H, KVH, HD = 32, 32, 8, 128
GQ = H // KVH            # query heads per kv head
BS = 16                  # kv-cache block size
MAX_KV = 4096
SCALE = 1.0 / float(np.sqrt(HD))
TSEQ = 512               # tokens per round
CHUNK = 128              # PV contraction chunk
NCORES = 8
NG = 4                   # sequence groups
GS = 8                   # sequences per group
GP = GQ * GS             # partitions per group (32)
NEG = -1.0e30
BF16 = ml_dtypes.bfloat16

_GRAPH_CACHE = {}
KV_BUFS = 3
WARM_N = 0  # PE warm-keeper dummy matmuls per round (HAM anti-throttle)


class _SplitDrainTileContext(tile.TileContext):
    """The walrus build in this container rejects >1 sync-wait on a CTRL
    (Drain) instruction; move the tail-drain's extra waits onto explicit
    single-sem wait instructions."""

    def _drain_and_barrier(self, tick_clock, wait_clock):
        import bass_rust as _br
        drain_inst = self.nc.sync.drain()
        wait_clock.add_sem_waits(
            drain_inst.ins, ScopedClock({None: tick_clock.global_clock}))
        si = drain_inst.ins.sync_info
        waits = list(si.on_wait) if si is not None else []
        if len(waits) > 1:
            si.on_wait = waits[:1]
            for w in waits[1:]:
                extra = self.nc.sync.drain()
                extra.ins.sync_info = _br.SyncInfo(on_wait=[w], on_update=[])
        self.nc.all_engine_barrier()
        assert self.sems is not None
        popped = self.nc._tile_sem_poison_stack.pop()
        assert popped is self._sem_poison
        self.nc.clear_and_free_semaphores(list(self.sems.allocated().values()))
        self.nc.all_engine_barrier()


def _split_multiwait_insts(nc):
    """This container's walrus accepts only ONE sync-wait per instruction.
    Tile's stage-1B can attach several; move the extras onto engine NOPs
    inserted immediately before the offending instruction."""
    import bass_rust as _br
    for f in nc.m.functions:
        for bb in f.blocks:
            il = list(bb.instructions)
            need = sum(
                max(0, len(i.sync_info.on_wait) - 1)
                for i in il if i.sync_info is not None)
            if not need:
                continue
            out = []
            for inst in il:
                si = inst.sync_info
                if si is not None and len(si.on_wait) > 1:
                    waits = list(si.on_wait)
                    si.on_wait = [waits[-1]]
                    for w in waits[:-1]:
                        nop = mybir.InstNoOp(
                            name=f"mwnop-{nc.next_id()}",
                            engine=inst.engine, ins=[], outs=[])
                        nop.sync_info = _br.SyncInfo(on_wait=[w], on_update=[])
                        nc.register_instruction(nop)
                        out.append(nop)
                out.append(inst)
            bb.instructions = out


def _plan(seqlens):
    sl = np.asarray(seqlens, dtype=np.int64)
    order = np.argsort(-sl, kind="stable")
    lens = sl[order]
    rounds = np.maximum((lens + TSEQ - 1) // TSEQ, 1).astype(np.int64)
    rg = [int(rounds[g * GS]) for g in range(NG)]
    act = [
        [int(np.sum(rounds[g * GS:(g + 1) * GS] > r)) for r in range(rg[g])]
        for g in range(NG)
    ]
    return order, lens, rounds, rg, act


def _build(rg, act, reps=1, parts="full"):
    """Build the SPMD graph. rg[g] = rounds in group g; act[g][r] = number of
    active (still-running) sequences of group g in round r.
    parts: 'full' | 'dma' (DMAs only) | 'scores' (no transpose/PV)."""
    tot = sum(TSEQ * a for g in range(NG) for a in act[g])
    DT = mybir.dt.bfloat16
    F32 = mybir.dt.float32
    nc = bass.Bass()
    kt_e = nc.declare_dram_parameter("kt", [HD, tot], DT, isOutput=False)
    v_e = nc.declare_dram_parameter("v", [tot, HD], DT, isOutput=False)
    # Block-column-expanded q: for sorted seq S, columns [GP*S, GP*S+GP) hold
    # q for its 4 heads at in-group offset 4*(S%GS), zeros elsewhere. PSUM
    # writes must start at partition 0/32/64, so per-seq score matmuls write
    # the full 32-row group tile and accumulate; the zero columns make the
    # other sequences' rows contribute nothing.
    q_e = nc.declare_dram_parameter("q", [HD, B * GP], DT, isOutput=False)
    m_e = nc.declare_dram_parameter("mask", [B * GQ, MAX_KV], F32, isOutput=False)
    o_e = nc.declare_dram_parameter("out", [B * GQ, HD], F32, isOutput=True)
    id_d = nc.inline_tensor(np.eye(GP, dtype=BF16), name="ident32")
    idf_d = nc.inline_tensor(np.eye(HD, dtype=np.float32), name="ident128")

    # per-seq round counts (for PV stop flags): rounds_j = #{r: act[g][r] > j}
    nrounds = [[sum(1 for r in range(rg[g]) if act[g][r] > j) for j in range(GS)]
               for g in range(NG)]

    with _SplitDrainTileContext(nc) as tc, ExitStack() as ctx:
        kp = ctx.enter_context(tc.tile_pool(name="k", bufs=KV_BUFS))
        vp = ctx.enter_context(tc.tile_pool(name="v", bufs=KV_BUFS))
        sp = ctx.enter_context(tc.tile_pool(name="scores", bufs=2))
        pp = ctx.enter_context(tc.tile_pool(name="p", bufs=2))
        mp = ctx.enter_context(tc.tile_pool(name="mask", bufs=2))
        ptp = ctx.enter_context(tc.tile_pool(name="pt", bufs=3))
        qp = ctx.enter_context(tc.tile_pool(name="q", bufs=1))
        op = ctx.enter_context(tc.tile_pool(name="o", bufs=2))
        st = ctx.enter_context(tc.tile_pool(name="st", bufs=6))
        psc = ctx.enter_context(tc.tile_pool(name="psc", bufs=2, space="PSUM"))
        pst = ctx.enter_context(tc.tile_pool(name="pst", bufs=2, space="PSUM"))
        psv = ctx.enter_context(tc.tile_pool(name="psv", bufs=2, space="PSUM"))
        pso = ctx.enter_context(tc.tile_pool(name="pso", bufs=1, space="PSUM"))

        q_sb = qp.tile([HD, B * GP], DT)
        nc.gpsimd.dma_start(q_sb[:], q_e[:])
        ident = qp.tile([GP, GP], DT)
        nc.gpsimd.dma_start(ident[:], id_d[:])
        identf = qp.tile([HD, HD], F32)
        nc.gpsimd.dma_start(identf[:], idf_d[:])

        if WARM_N:
            psw = ctx.enter_context(tc.tile_pool(name="psw", bufs=1, space="PSUM"))
            warm_ps = psw.tile([GP, TSEQ], F32)

        def emit_warm(n):
            # HAM anti-throttle: keep the PE busy through DMA-wait gaps with
            # dummy matmuls on always-resident data so the 4096-cycle activity
            # window never sees an idle period and the clock stays at 2.4 GHz
            for _ in range(n):
                nc.tensor.matmul(
                    warm_ps[:, :], q_sb[:, 0:GP],
                    q_sb[:, 0:TSEQ], start=True, stop=True,
                    skip_group_check=True)

        for _ in range(reps):
            offs = {"k": 0, "v": 0}
            stg = [dict() for _ in range(NG)]

            def emit_a(g):
                """scores: K DMAs + masked matmuls, fused running row-max."""
                R = rg[g]
                Lg = TSEQ * R
                s = stg[g]
                sc_t = sp.tile([GP, Lg], F32, tag="sc")
                p_t = pp.tile([GP, Lg], DT, tag="p")
                s["sc"] = sc_t
                s["p"] = p_t
                mk_t = mp.tile([GP, Lg], F32, tag="mk")
                if parts not in ("kdma", "vdma", "noop"):
                    nc.sync.dma_start(mk_t[:], m_e[g * GP:(g + 1) * GP, 0:Lg])
                for r in range(R):
                    a = act[g][r]
                    w = TSEQ * a
                    if parts not in ("vdma", "noop"):
                        kt_t = kp.tile([HD, w], DT, tag="k")
                        nc.sync.dma_start(kt_t[:], kt_e[:, offs["k"]:offs["k"] + w])
                    if parts in ("full", "scores"):
                        ps_t = psc.tile([GP, TSEQ], F32, tag="ps")
                        for j in range(a):
                            S = g * GS + j
                            nc.tensor.matmul(
                                ps_t[:, :],
                                q_sb[:, GP * S:GP * (S + 1)],
                                kt_t[:, TSEQ * j:TSEQ * (j + 1)],
                                start=(j == 0), stop=(j == a - 1))
                        nc.vector.tensor_add(
                            s["sc"][:, TSEQ * r:TSEQ * (r + 1)],
                            ps_t[:],
                            mk_t[:, TSEQ * r:TSEQ * (r + 1)])
                        if WARM_N:
                            emit_warm(WARM_N)
                    offs["k"] += w

            def emit_b(g):
                """softmax: bias, exp (fused row-sum), reciprocal."""
                if parts != "full":
                    return
                s = stg[g]
                mx_t = st.tile([GP, 1], F32, tag="mx")
                nc.vector.reduce_max(mx_t[:], s["sc"][:], axis=mybir.AxisListType.X)
                b_t = st.tile([GP, 1], F32, tag="b")
                nc.scalar.mul(b_t[:], mx_t[:], -SCALE)
                s_t = st.tile([GP, 1], F32, tag="s")
                nc.scalar.activation(
                    s["p"][:], s["sc"][:], mybir.ActivationFunctionType.Exp,
                    bias=b_t[:], scale=SCALE, accum_out=s_t[:])
                r_t = st.tile([GP, 1], F32, tag="r")
                nc.vector.reciprocal(r_t[:], s_t[:])
                s["r"] = r_t

            def emit_c(g):
                """PV: V DMAs, p transposes, accumulated p^T@V, normalize."""
                R = rg[g]
                s = stg[g]
                if parts == "full":
                    pv_t = psv.tile([HD, GP], F32, tag="pv")
                for r in range(R):
                    a = act[g][r]
                    w = TSEQ * a
                    # v_e rows within a round block are host-permuted to
                    # (p, jc): row = voff + p*(4a) + jc, so each partition
                    # reads one contiguous 4a*HD run (big DMA descriptors)
                    if parts not in ("kdma", "noop"):
                        v_t = vp.tile([CHUNK, GQ * a, HD], DT, tag="v")
                        nc.scalar.dma_start(
                            v_t[:],
                            v_e[offs["v"]:offs["v"] + w, :].rearrange(
                                "(p c) d -> p c d", p=CHUNK))
                    if parts == "full":
                        pt_ps = pst.tile([CHUNK, 4 * GP], DT, tag="ptps")
                        for c in range(4):
                            nc.tensor.transpose(
                                pt_ps[:, GP * c:GP * (c + 1)],
                                s["p"][:, TSEQ * r + CHUNK * c:
                                       TSEQ * r + CHUNK * (c + 1)],
                                ident[:])
                        pt_sb = ptp.tile([CHUNK, 4 * GP], DT, tag="pt")
                        nc.vector.tensor_copy(pt_sb[:], pt_ps[:])
                        # single accumulation group for the whole tile:
                        # start=True clears has_written for the entire BANK, so
                        # per-sequence groups would clobber sibling columns
                        for j in range(a):
                            for c in range(4):
                                nc.tensor.matmul(
                                    pv_t[:, GQ * j:GQ * (j + 1)],
                                    v_t[:, GQ * j + c, :],
                                    pt_sb[:, GP * c + GQ * j:GP * c + GQ * (j + 1)],
                                    start=(r == 0 and c == 0 and j == 0),
                                    stop=(r == R - 1 and c == 3 and j == a - 1),
                                    skip_group_check=True)
                        if WARM_N:
                            emit_warm(WARM_N)
                    offs["v"] += w
                # transpose (HD, GP) -> (GP, HD), normalize, store
                if parts == "full":
                    ot_sb = op.tile([HD, GP], F32, tag="ot")
                    nc.vector.tensor_copy(ot_sb[:], pv_t[:])
                    otr_ps = pso.tile([GP, HD], F32, tag="otr")
                    nc.tensor.transpose(otr_ps[:], ot_sb[:], identf[:])
                    o_t = op.tile([GP, HD], F32, tag="o")
                    nc.vector.tensor_scalar_mul(o_t[:], otr_ps[:], s["r"][:])
                    nc.gpsimd.dma_start(o_e[g * GP:(g + 1) * GP, :], o_t[:])

            # software-pipeline the groups: emit group g+1's scores before
            # group g's PV so the PE never stalls on the softmax chain
            emit_a(0)
            for g in range(NG):
                if g + 1 < NG:
                    emit_a(g + 1)
                emit_b(g)
                emit_c(g)
    _split_multiwait_insts(nc)
    return nc, tot


def _pack(inputs, order, lens, rounds, rg, act, tot):
    k_cache = np.asarray(inputs["k_cache"])
    v_cache = np.asarray(inputs["v_cache"])
    bt = np.asarray(inputs["block_tables"]).astype(np.int64)
    q = np.asarray(inputs["q"], dtype=np.float32)

    kt = np.zeros((NCORES, HD, tot), dtype=BF16)
    vb = np.zeros((NCORES, tot, HD), dtype=BF16)
    off = 0
    for g in range(NG):
        for r in range(rg[g]):
            a = act[g][r]
            # view of this round's V block: (cores, p=128, jc=4a, HD) — rows
            # permuted so each SBUF partition p reads one contiguous run
            vblk = vb[:, off:off + TSEQ * a, :].reshape(NCORES, CHUNK, GQ * a, HD)
            for j in range(a):
                S = g * GS + j
                o = int(order[S])
                t0 = r * TSEQ
                t1 = min(int(lens[S]), t0 + TSEQ)
                nt = t1 - t0
                b0 = t0 // BS
                b1 = (t1 + BS - 1) // BS
                blocks = bt[o, b0:b1]
                kk = k_cache[blocks].reshape(-1, KVH, HD)[:nt]  # (nt, KVH, HD)
                vv = v_cache[blocks].reshape(-1, KVH, HD)[:nt]
                kt[:, :, off + TSEQ * j:off + TSEQ * j + nt] = kk.transpose(1, 2, 0)
                # scatter vv tokens t = c*CHUNK + p into vblk[:, p, 4j+c, :]
                vvp = np.zeros((TSEQ, KVH, HD), dtype=vv.dtype)
                vvp[:nt] = vv
                vvp = vvp.reshape(GQ, CHUNK, KVH, HD)          # (c, p, kvh, d)
                vblk[:, :, GQ * j:GQ * (j + 1), :] = vvp.transpose(2, 1, 0, 3)
            off += TSEQ * a

    qs = q[order].reshape(B, KVH, GQ, HD)
    # block-column-expanded q: (KVH, HD, B*GP); for sorted seq S the 4 head
    # columns sit at GP*S + GQ*(S % GS), everything else stays zero.
    qb = np.zeros((KVH, HD, B * GP), dtype=BF16)
    for S in range(B):
        col = GP * S + GQ * (S % GS)
        qb[:, :, col:col + GQ] = qs[S].transpose(0, 2, 1).astype(BF16)
    lens_rep = np.repeat(lens, GQ)
    mask = np.where(np.arange(MAX_KV)[None, :] < lens_rep[:, None],
                    np.float32(0.0), np.float32(NEG)).astype(np.float32)
    in_maps = [
        {"kt": np.ascontiguousarray(kt[c]),
         "v": np.ascontiguousarray(vb[c]),
         "q": np.ascontiguousarray(qb[c]),
         "mask": mask}
        for c in range(NCORES)
    ]
    return in_maps


def kernel(**inputs) -> np.ndarray:
    seqlens = np.asarray(inputs["cache_seqlens"]).astype(np.int64)
    order, lens, rounds, rg, act = _plan(seqlens)
    key = (tuple(rg), tuple(tuple(a) for a in act))
    if key not in _GRAPH_CACHE:
        _GRAPH_CACHE[key] = _build(rg, act)
    nc, tot = _GRAPH_CACHE[key]
    in_maps = _pack(inputs, order, lens, rounds, rg, act, tot)
    res = run_bass_kernel_spmd(nc, in_maps, core_ids=list(range(NCORES))).results
    out = np.empty((B, H, HD), dtype=np.float32)
    for c in range(NCORES):
        oc = np.asarray(res[c]["out"]).reshape(B, GQ, HD)
        out[order, GQ * c:GQ * (c + 1), :] = oc
    return out


# revision 42
# speedup vs baseline: 1.0325x; 1.0325x over previous
"""Paged-attention decode (GQA, B=32, H=32, KVH=8, HD=128) on 8 TRN2 NeuronCores.

Sharding: tensor-parallel over KV heads (8 KV heads -> 8 cores). Each core
handles one KV head and its 4 query heads for ALL 32 sequences, so the
per-core work is identical regardless of the sequence-length distribution.

Host side (not on the device critical path):
  - gathers each sequence's valid KV-cache blocks via block_tables,
  - sorts sequences by length (desc) and packs them into 4 groups of 8,
  - packs K transposed as (HD, tokens) and V as (tokens, HD), bf16,
    round-major (one contiguous ~1 MiB DMA per 512-token round),
  - builds an additive -1e30 length mask.

Device kernel (per core, identical SPMD graph):
  for each group of 8 seqs:
    phase A: per 512-token round: 1 K DMA + 8 q@K^T matmuls -> PSUM,
             PSUM + mask -> SBUF scores
    phase B: row-max, exp (fused bias/scale/row-sum) -> p (bf16)
    phase C: per round: 1 V DMA, PE-transpose p chunks, p^T @ V accumulated
             in PSUM per sequence
    normalize by 1/row-sum and DMA out (seq*qhead, HD).
"""

from contextlib import ExitStack

import numpy as np
import ml_dtypes

import concourse.bass as bass
import concourse.tile as tile
from concourse import mybir
from concourse.bass_utils import run_bass_kernel_spmd
from concourse.vector_clock import ScopedClock

B, H, KVH, HD = 32, 32, 8, 128
GQ = H // KVH            # query heads per kv head
BS = 16                  # kv-cache block size
MAX_KV = 4096
SCALE = 1.0 / float(np.sqrt(HD))
TSEQ = 512               # tokens per round
CHUNK = 128              # PV contraction chunk
NCORES = 8
NG = 4                   # sequence groups
GS = 8                   # sequences per group
GP = GQ * GS             # partitions per group (32)
NEG = -1.0e30
BF16 = ml_dtypes.bfloat16

_GRAPH_CACHE = {}
KV_BUFS = 3
WARM_N = 0  # PE warm-keeper dummy matmuls per round (HAM anti-throttle)


class _SplitDrainTileContext(tile.TileContext):
    """The walrus build in this container rejects >1 sync-wait on a CTRL
    (Drain) instruction; move the tail-drain's extra waits onto explicit
    single-sem wait instructions."""

    def _drain_and_barrier(self, tick_clock, wait_clock):
        import bass_rust as _br
        drain_inst = self.nc.sync.drain()
        wait_clock.add_sem_waits(
            drain_inst.ins, ScopedClock({None: tick_clock.global_clock}))
        si = drain_inst.ins.sync_info
        waits = list(si.on_wait) if si is not None else []
        if len(waits) > 1:
            si.on_wait = waits[:1]
            for w in waits[1:]:
                extra = self.nc.sync.drain()
                extra.ins.sync_info = _br.SyncInfo(on_wait=[w], on_update=[])
        self.nc.all_engine_barrier()
        assert self.sems is not None
        popped = self.nc._tile_sem_poison_stack.pop()
        assert popped is self._sem_poison
        self.nc.clear_and_free_semaphores(list(self.sems.allocated().values()))
        self.nc.all_engine_barrier()


def _split_multiwait_insts(nc):
    """This container's walrus accepts only ONE sync-wait per instruction.
    Tile's stage-1B can attach several; move the extras onto engine NOPs
    inserted immediately before the offending instruction."""
    import bass_rust as _br
    for f in nc.m.functions:
        for bb in f.blocks:
            il = list(bb.instructions)
            need = sum(
                max(0, len(i.sync_info.on_wait) - 1)
                for i in il if i.sync_info is not None)
            if not need:
                continue
            out = []
            for inst in il:
                si = inst.sync_info
                if si is not None and len(si.on_wait) > 1:
                    waits = list(si.on_wait)
                    si.on_wait = [waits[-1]]
                    for w in waits[:-1]:
                        nop = mybir.InstNoOp(
                            name=f"mwnop-{nc.next_id()}",
                            engine=inst.engine, ins=[], outs=[])
                        nop.sync_info = _br.SyncInfo(on_wait=[w], on_update=[])
                        nc.register_instruction(nop)
                        out.append(nop)
                out.append(inst)
            bb.instructions = out


def _plan(seqlens):
    sl = np.asarray(seqlens, dtype=np.int64)
    order = np.argsort(-sl, kind="stable")
    lens = sl[order]
    rounds = np.maximum((lens + TSEQ - 1) // TSEQ, 1).astype(np.int64)
    rg = [int(rounds[g * GS]) for g in range(NG)]
    act = [
        [int(np.sum(rounds[g * GS:(g + 1) * GS] > r)) for r in range(rg[g])]
        for g in range(NG)
    ]
    return order, lens, rounds, rg, act


def _build(rg, act, reps=1, parts="full"):
    """Build the SPMD graph. rg[g] = rounds in group g; act[g][r] = number of
    active (still-running) sequences of group g in round r.
    parts: 'full' | 'dma' (DMAs only) | 'scores' (no transpose/PV)."""
    tot = sum(TSEQ * a for g in range(NG) for a in act[g])
    DT = mybir.dt.bfloat16
    F32 = mybir.dt.float32
    nc = bass.Bass()
    kt_e = nc.declare_dram_parameter("kt", [HD, tot], DT, isOutput=False)
    v_e = nc.declare_dram_parameter("v", [tot, HD], DT, isOutput=False)
    # Block-column-expanded q: for sorted seq S, columns [GP*S, GP*S+GP) hold
    # q for its 4 heads at in-group offset 4*(S%GS), zeros elsewhere. PSUM
    # writes must start at partition 0/32/64, so per-seq score matmuls write
    # the full 32-row group tile and accumulate; the zero columns make the
    # other sequences' rows contribute nothing.
    q_e = nc.declare_dram_parameter("q", [HD, B * GP], DT, isOutput=False)
    m_e = nc.declare_dram_parameter("mask", [B * GQ, MAX_KV], F32, isOutput=False)
    o_e = nc.declare_dram_parameter("out", [B * GQ, HD], F32, isOutput=True)
    id_d = nc.inline_tensor(np.eye(GP, dtype=BF16), name="ident32")
    idf_d = nc.inline_tensor(np.eye(HD, dtype=np.float32), name="ident128")

    # per-seq round counts (for PV stop flags): rounds_j = #{r: act[g][r] > j}
    nrounds = [[sum(1 for r in range(rg[g]) if act[g][r] > j) for j in range(GS)]
               for g in range(NG)]

    with _SplitDrainTileContext(nc) as tc, ExitStack() as ctx:
        kp = ctx.enter_context(tc.tile_pool(name="k", bufs=KV_BUFS))
        vp = ctx.enter_context(tc.tile_pool(name="v", bufs=KV_BUFS))
        sp = ctx.enter_context(tc.tile_pool(name="scores", bufs=2))
        pp = ctx.enter_context(tc.tile_pool(name="p", bufs=2))
        mp = ctx.enter_context(tc.tile_pool(name="mask", bufs=2))
        ptp = ctx.enter_context(tc.tile_pool(name="pt", bufs=3))
        qp = ctx.enter_context(tc.tile_pool(name="q", bufs=1))
        op = ctx.enter_context(tc.tile_pool(name="o", bufs=2))
        st = ctx.enter_context(tc.tile_pool(name="st", bufs=6))
        psc = ctx.enter_context(tc.tile_pool(name="psc", bufs=2, space="PSUM"))
        pst = ctx.enter_context(tc.tile_pool(name="pst", bufs=2, space="PSUM"))
        psv = ctx.enter_context(tc.tile_pool(name="psv", bufs=2, space="PSUM"))
        pso = ctx.enter_context(tc.tile_pool(name="pso", bufs=1, space="PSUM"))

        q_sb = qp.tile([HD, B * GP], DT)
        nc.gpsimd.dma_start(q_sb[:], q_e[:])
        ident = qp.tile([GP, GP], DT)
        nc.gpsimd.dma_start(ident[:], id_d[:])
        identf = qp.tile([HD, HD], F32)
        nc.gpsimd.dma_start(identf[:], idf_d[:])

        if WARM_N:
            psw = ctx.enter_context(tc.tile_pool(name="psw", bufs=1, space="PSUM"))
            warm_ps = psw.tile([GP, TSEQ], F32)

        def emit_warm(n):
            # HAM anti-throttle: keep the PE busy through DMA-wait gaps with
            # dummy matmuls on always-resident data so the 4096-cycle activity
            # window never sees an idle period and the clock stays at 2.4 GHz
            for _ in range(n):
                nc.tensor.matmul(
                    warm_ps[:, :], q_sb[:, 0:GP],
                    q_sb[:, 0:TSEQ], start=True, stop=True,
                    skip_group_check=True)

        for _ in range(reps):
            offs = {"k": 0, "v": 0}
            stg = [dict() for _ in range(NG)]

            def emit_a(g):
                """scores: K DMAs + masked matmuls, fused running row-max."""
                R = rg[g]
                Lg = TSEQ * R
                s = stg[g]
                sc_t = sp.tile([GP, Lg], F32, tag="sc")
                p_t = pp.tile([GP, Lg], DT, tag="p")
                s["sc"] = sc_t
                s["p"] = p_t
                mk_t = mp.tile([GP, Lg], F32, tag="mk")
                if parts not in ("kdma", "vdma", "noop"):
                    nc.sync.dma_start(mk_t[:], m_e[g * GP:(g + 1) * GP, 0:Lg])
                for r in range(R):
                    a = act[g][r]
                    w = TSEQ * a
                    if parts not in ("vdma", "noop"):
                        kt_t = kp.tile([HD, w], DT, tag="k")
                        nc.sync.dma_start(kt_t[:], kt_e[:, offs["k"]:offs["k"] + w])
                    if parts in ("full", "scores"):
                        ps_t = psc.tile([GP, TSEQ], F32, tag="ps")
                        for j in range(a):
                            S = g * GS + j
                            nc.tensor.matmul(
                                ps_t[:, :],
                                q_sb[:, GP * S:GP * (S + 1)],
                                kt_t[:, TSEQ * j:TSEQ * (j + 1)],
                                start=(j == 0), stop=(j == a - 1))
                        nc.vector.tensor_add(
                            s["sc"][:, TSEQ * r:TSEQ * (r + 1)],
                            ps_t[:],
                            mk_t[:, TSEQ * r:TSEQ * (r + 1)])
                        if WARM_N:
                            emit_warm(WARM_N)
                    offs["k"] += w

            def emit_b(g):
                """softmax: bias, exp (fused row-sum), reciprocal."""
                if parts != "full":
                    return
                s = stg[g]
                mx_t = st.tile([GP, 1], F32, tag="mx")
                nc.vector.reduce_max(mx_t[:], s["sc"][:], axis=mybir.AxisListType.X)
                b_t = st.tile([GP, 1], F32, tag="b")
                nc.scalar.mul(b_t[:], mx_t[:], -SCALE)
                s_t = st.tile([GP, 1], F32, tag="s")
                nc.scalar.activation(
                    s["p"][:], s["sc"][:], mybir.ActivationFunctionType.Exp,
                    bias=b_t[:], scale=SCALE, accum_out=s_t[:])
                r_t = st.tile([GP, 1], F32, tag="r")
                nc.vector.reciprocal(r_t[:], s_t[:])
                s["r"] = r_t

            def emit_c(g):
                """PV: V DMAs, p transposes, accumulated p^T@V, normalize."""
                R = rg[g]
                s = stg[g]
                if parts == "full":
                    pv_t = psv.tile([HD, GP], F32, tag="pv")
                for r in range(R):
                    a = act[g][r]
                    w = TSEQ * a
                    # v_e rows within a round block are host-permuted to
                    # (p, jc): row = voff + p*(4a) + jc, so each partition
                    # reads one contiguous 4a*HD run (big DMA descriptors)
                    if parts not in ("kdma", "noop"):
                        v_t = vp.tile([CHUNK, GQ * a, HD], DT, tag="v")
                        nc.sync.dma_start(
                            v_t[:],
                            v_e[offs["v"]:offs["v"] + w, :].rearrange(
                                "(p c) d -> p c d", p=CHUNK))
                    if parts == "full":
                        pt_ps = pst.tile([CHUNK, 4 * GP], DT, tag="ptps")
                        for c in range(4):
                            nc.tensor.transpose(
                                pt_ps[:, GP * c:GP * (c + 1)],
                                s["p"][:, TSEQ * r + CHUNK * c:
                                       TSEQ * r + CHUNK * (c + 1)],
                                ident[:])
                        pt_sb = ptp.tile([CHUNK, 4 * GP], DT, tag="pt")
                        nc.vector.tensor_copy(pt_sb[:], pt_ps[:])
                        # single accumulation group for the whole tile:
                        # start=True clears has_written for the entire BANK, so
                        # per-sequence groups would clobber sibling columns
                        for j in range(a):
                            for c in range(4):
                                nc.tensor.matmul(
                                    pv_t[:, GQ * j:GQ * (j + 1)],
                                    v_t[:, GQ * j + c, :],
                                    pt_sb[:, GP * c + GQ * j:GP * c + GQ * (j + 1)],
                                    start=(r == 0 and c == 0 and j == 0),
                                    stop=(r == R - 1 and c == 3 and j == a - 1),
                                    skip_group_check=True)
                        if WARM_N:
                            emit_warm(WARM_N)
                    offs["v"] += w
                # transpose (HD, GP) -> (GP, HD), normalize, store
                if parts == "full":
                    ot_sb = op.tile([HD, GP], F32, tag="ot")
                    nc.vector.tensor_copy(ot_sb[:], pv_t[:])
                    otr_ps = pso.tile([GP, HD], F32, tag="otr")
                    nc.tensor.transpose(otr_ps[:], ot_sb[:], identf[:])
                    o_t = op.tile([GP, HD], F32, tag="o")
                    nc.vector.tensor_scalar_mul(o_t[:], otr_ps[:], s["r"][:])
                    nc.gpsimd.dma_start(o_e[g * GP:(g + 1) * GP, :], o_t[:])

            # software-pipeline the groups: emit group g+1's scores before
            # group g's PV so the PE never stalls on the softmax chain
            emit_a(0)
            for g in range(NG):
                if g + 1 < NG:
                    emit_a(g + 1)
                emit_b(g)
                emit_c(g)
    _split_multiwait_insts(nc)
    return nc, tot


def _pack(inputs, order, lens, rounds, rg, act, tot):
    k_cache = np.asarray(inputs["k_cache"])
    v_cache = np.asarray(inputs["v_cache"])
    bt = np.asarray(inputs["block_tables"]).astype(np.int64)
    q = np.asarray(inputs["q"], dtype=np.float32)

    kt = np.zeros((NCORES, HD, tot), dtype=BF16)
    vb = np.zeros((NCORES, tot, HD), dtype=BF16)
    off = 0
    for g in range(NG):
        for r in range(rg[g]):
            a = act[g][r]
            # view of this round's V block: (cores, p=128, jc=4a, HD) — rows
            # permuted so each SBUF partition p reads one contiguous run
            vblk = vb[:, off:off + TSEQ * a, :].reshape(NCORES, CHUNK, GQ * a, HD)
            for j in range(a):
                S = g * GS + j
                o = int(order[S])
                t0 = r * TSEQ
                t1 = min(int(lens[S]), t0 + TSEQ)
                nt = t1 - t0
                b0 = t0 // BS
                b1 = (t1 + BS - 1) // BS
                blocks = bt[o, b0:b1]
                kk = k_cache[blocks].reshape(-1, KVH, HD)[:nt]  # (nt, KVH, HD)
                vv = v_cache[blocks].reshape(-1, KVH, HD)[:nt]
                kt[:, :, off + TSEQ * j:off + TSEQ * j + nt] = kk.transpose(1, 2, 0)
                # scatter vv tokens t = c*CHUNK + p into vblk[:, p, 4j+c, :]
                vvp = np.zeros((TSEQ, KVH, HD), dtype=vv.dtype)
                vvp[:nt] = vv
                vvp = vvp.reshape(GQ, CHUNK, KVH, HD)          # (c, p, kvh, d)
                vblk[:, :, GQ * j:GQ * (j + 1), :] = vvp.transpose(2, 1, 0, 3)
            off += TSEQ * a

    qs = q[order].reshape(B, KVH, GQ, HD)
    # block-column-expanded q: (KVH, HD, B*GP); for sorted seq S the 4 head
    # columns sit at GP*S + GQ*(S % GS), everything else stays zero.
    qb = np.zeros((KVH, HD, B * GP), dtype=BF16)
    for S in range(B):
        col = GP * S + GQ * (S % GS)
        qb[:, :, col:col + GQ] = qs[S].transpose(0, 2, 1).astype(BF16)
    lens_rep = np.repeat(lens, GQ)
    mask = np.where(np.arange(MAX_KV)[None, :] < lens_rep[:, None],
                    np.float32(0.0), np.float32(NEG)).astype(np.float32)
    in_maps = [
        {"kt": np.ascontiguousarray(kt[c]),
         "v": np.ascontiguousarray(vb[c]),
         "q": np.ascontiguousarray(qb[c]),
         "mask": mask}
        for c in range(NCORES)
    ]
    return in_maps


def kernel(**inputs) -> np.ndarray:
    seqlens = np.asarray(inputs["cache_seqlens"]).astype(np.int64)
    order, lens, rounds, rg, act = _plan(seqlens)
    key = (tuple(rg), tuple(tuple(a) for a in act))
    if key not in _GRAPH_CACHE:
        _GRAPH_CACHE[key] = _build(rg, act)
    nc, tot = _GRAPH_CACHE[key]
    in_maps = _pack(inputs, order, lens, rounds, rg, act, tot)
    res = run_bass_kernel_spmd(nc, in_maps, core_ids=list(range(NCORES))).results
    out = np.empty((B, H, HD), dtype=np.float32)
    for c in range(NCORES):
        oc = np.asarray(res[c]["out"]).reshape(B, GQ, HD)
        out[order, GQ * c:GQ * (c + 1), :] = oc
    return out
